# revision 12
# baseline (speedup 1.0000x reference)
"""Trainium2 Bass kernel for nn_BigramLanguageModel_67044439490742.

6-layer causal transformer: B=2, T=1024, C=1024, H=16 heads, FF=4096,
V=32000, f32 I/O.

Sharding over 8 NeuronCores: DP2 over batch x sequence-parallel-4 within
each batch group. Token rows split into 8 strips of 128 (per batch); core q
of a group owns global strips {q, 7-q}, stored contiguously at block
positions {2q, 2q+1} of the strip-permuted layout perm=[0,7,1,6,2,5,3,4]
(AllGather rank-order concat reproduces exactly this layout). Per layer:
LN1 + K/V projections run replicated over the full batch, then
Q/attention/out-proj/FFN only for the core's own 256 tokens, then one
AllGather of the updated strips. The residual stream for owned strips
lives in f32 SBUF ("xlocal"); the AllGather replica only feeds LN1->K,V
and the final LN->lm_head. lm_head is sharded over vocab (8000 cols/core).

The SPMD program is identical on all cores: per-core causality is encoded
in shipped additive-mask inputs (strip slot A covers k-blocks 0..3, slot B
0..7; non-causal blocks get -1e30), and the lm_head shard is per-core
data.

Precision strategy (v4): this net amplifies any trunk-activation noise
~20x into next-layer attention scores (std ~19 -> exp), so NO trunk
tensor may be stored bf16 (measured 5-9e-2 rel err per bf16 site).
Everything is stored as full f32 bits; all large GEMMs (N>=256) run as
fp32r -- full-rate PE streaming over f32 bits, measured to add only
~1e-3-grade noise (vs fp32's 4 cyc/row 2-pass). Transposes stay true
fp32. Only the lm_head (xfT + lmw) is bf16: its noise hits logits
directly without amplification (~2e-3). LN gammas (and the C**0.25 qk
scale) are folded into downstream weights on the host; all reference
biases/betas are structurally zero and skipped on device.

V / relu(FFN1) / K^T stage through DRAM. wo and w2 are loaded exactly
once per layer (w2 via a half-column pass reused for both strips).
"""

import numpy as np

B, T, C, H, L, V = 2, 1024, 1024, 16, 6, 32000
HS = C // H          # 64
FF = 4 * C           # 4096
SCALE = float(C) ** 0.25
NB = 8               # strips of 128 per batch
P = 128
NCORES = 8
GSIZE = 4
VSH = V // GSIZE     # 8000
NCHUNK = 16
VCH = VSH // NCHUNK  # 500
PERM = [0, 7, 1, 6, 2, 5, 3, 4]
POS = [PERM.index(s) for s in range(NB)]
NEG = -1.0e30
EPS = 1e-5

_CACHE = {}


def _build_program(nl=L):
    import concourse.bacc as bacc
    import concourse.mybir as mybir
    import concourse.tile as tile

    F32 = mybir.dt.float32
    F32R = mybir.dt.float32r
    BF16 = mybir.dt.bfloat16
    AF = mybir.ActivationFunctionType
    ALU = mybir.AluOpType
    AX = mybir.AxisListType

    nc = bacc.Bacc(None, target_bir_lowering=False)

    x0 = nc.declare_dram_parameter("x0", [NB, P, C], F32, isOutput=False)
    xloc0 = nc.declare_dram_parameter("xloc0", [2, P, C], F32, isOutput=False)
    wq = nc.declare_dram_parameter("wq", [L, 8, P, 8, P], F32, isOutput=False)
    wk = nc.declare_dram_parameter("wk", [L, 8, P, 8, P], F32, isOutput=False)
    wv = nc.declare_dram_parameter("wv", [L, 8, P, C], F32R, isOutput=False)
    wo = nc.declare_dram_parameter("wo", [L, 8, P, C], F32, isOutput=False)
    # w1 grouped per ff column block: [L, 32 fb, 128 ci, 8 cc, 128 fi]
    w1 = nc.declare_dram_parameter("w1", [L, 32, P, 8, P], F32R, isOutput=False)
    w2 = nc.declare_dram_parameter("w2", [L, 32, P, C], F32R, isOutput=False)
    bqs = nc.declare_dram_parameter("bqs", [L, P, 8], F32, isOutput=False)
    bks = nc.declare_dram_parameter("bks", [L, P, 8], F32, isOutput=False)
    b1s = nc.declare_dram_parameter("b1s", [L, P, 32], F32, isOutput=False)
    lmbr = nc.declare_dram_parameter("lmbr", [P, VSH], F32, isOutput=False)
    lmw = nc.declare_dram_parameter("lmw", [8, P, VSH], BF16, isOutput=False)
    maskA = nc.declare_dram_parameter("maskA", [P, 4 * P], F32, isOutput=False)
    maskB = nc.declare_dram_parameter("maskB", [P, 8 * P], F32, isOutput=False)
    ident = nc.declare_dram_parameter("ident", [P, P], F32, isOutput=False)
    identB = nc.declare_dram_parameter("identB", [P, P], BF16, isOutput=False)
    logits = nc.declare_dram_parameter("logits", [NB, P, VSH], F32, isOutput=True)

    with tile.TileContext(nc) as tc:
        with (
            tc.tile_pool(name="const", bufs=1) as cpool,
            tc.tile_pool(name="xlocal", bufs=1) as xlpool,
            tc.tile_pool(name="persist", bufs=1) as pers,
            tc.tile_pool(name="wres", bufs=1) as wres,
            tc.tile_pool(name="wstream", bufs=2) as wstr,
            tc.tile_pool(name="xstream", bufs=2) as xstr,
            tc.tile_pool(name="attn", bufs=2) as attn,
            tc.tile_pool(name="stat", bufs=4) as stat,
            tc.tile_pool(name="outst", bufs=3) as outst,
            tc.tile_pool(name="psB", bufs=1, space="PSUM") as psB,
            tc.tile_pool(name="psmm", bufs=2, space="PSUM") as psmm,
            tc.tile_pool(name="pstr", bufs=2, space="PSUM") as pstr,
            tc.tile_pool(name="psot", bufs=2, space="PSUM") as psot,
            tc.tile_pool(name="dram", bufs=2, space="DRAM") as dpool,
        ):
            ident_sb = cpool.tile([P, P], F32)
            nc.sync.dma_start(ident_sb[:], ident[:])
            identB_sb = cpool.tile([P, P], BF16)
            nc.sync.dma_start(identB_sb[:], identB[:])
            maskA_sb = cpool.tile([P, 4 * P], F32)
            nc.sync.dma_start(maskA_sb[:], maskA[:])
            maskB_sb = cpool.tile([P, 8 * P], F32)
            nc.sync.dma_start(maskB_sb[:], maskB[:])
            eps_sb = cpool.tile([P, 1], F32)
            nc.gpsimd.memset(eps_sb[:], EPS)

            xlocal = [xlpool.tile([P, C], F32, tag=f"xloc{i}", name=f"xloc{i}") for i in range(2)]
            nc.sync.dma_start(xlocal[0][:], xloc0[0])
            nc.sync.dma_start(xlocal[1][:], xloc0[1])

            def layernorm(src_ap, dst_ap):
                st = stat.tile([P, 12], F32, tag="bnst")
                nc.vector.bn_stats(st[:, 0:6], src_ap[:, 0:512])
                nc.vector.bn_stats(st[:, 6:12], src_ap[:, 512:1024])
                mv = stat.tile([P, 2], F32, tag="bnagg")
                nc.vector.bn_aggr(mv[:], st[:])
                sd = stat.tile([P, 1], F32, tag="sd")
                nc.scalar.activation(sd[:], mv[:, 1:2], AF.Sqrt, bias=eps_sb[:])
                rs = stat.tile([P, 1], F32, tag="rs")
                nc.vector.reciprocal(rs[:], sd[:])
                nc.vector.tensor_scalar(
                    out=dst_ap, in0=src_ap,
                    scalar1=mv[:, 0:1], scalar2=rs[:],
                    op0=ALU.subtract, op1=ALU.mult,
                )

            def transpose_block(dst_ap, src_ap, dt, id_sb, dst2_ap=None):
                pt = pstr.tile([P, P], dt, tag="tp")
                nc.tensor.transpose(pt[:], src_ap, id_sb[:])
                nc.vector.tensor_copy(dst_ap, pt[:])
                if dst2_ap is not None:
                    nc.scalar.copy(dst2_ap, pt[:])

            ag_out = None
            for l in range(nl):
                xsrc = x0 if l == 0 else ag_out

                hT = pers.tile([P, 8, NB * P], F32, tag="hT")
                hTr = pers.tile([P, 8, NB * P], F32R, tag="hTr")
                hTm = pers.tile([P, 8, 2 * P], F32, tag="hTm")
                qT = pers.tile([P, 8, 2 * P], F32, tag="qT")
                oT = pers.tile([P, 8, 2 * P], F32, tag="oT")
                h2T = pers.tile([P, 8, 2 * P], F32R, tag="h2T")
                kstage = dpool.tile([8, P, NB * P], F32, tag="kstage")
                vstage = dpool.tile([NB, P, C], F32, tag="vstage")
                ffstage = dpool.tile([32, P, 2 * P], F32R, tag="ffstage")

                # ---- LN1 full batch -> hT (transposed; GLOBAL col order) ----
                for rb in range(NB):
                    xr = xstr.tile([P, C], F32, tag="xrep")
                    nc.sync.dma_start(xr[:], xsrc[rb])
                    hl = xstr.tile([P, C], F32, tag="lnout")
                    layernorm(xr[:], hl[:])
                    gb = PERM[rb]  # global strip of this replica block
                    for cc in range(8):
                        transpose_block(hT[:, cc, gb * P:(gb + 1) * P],
                                        hl[:, cc * P:(cc + 1) * P],
                                        F32, ident_sb,
                                        dst2_ap=hTr[:, cc, gb * P:(gb + 1) * P])

                # ---- LN1 of owned strips -> hTm ----
                for i in range(2):
                    hlm = xstr.tile([P, C], F32, tag="lnout")
                    layernorm(xlocal[i][:], hlm[:])
                    for cc in range(8):
                        transpose_block(hTm[:, cc, i * P:(i + 1) * P],
                                        hlm[:, cc * P:(cc + 1) * P],
                                        F32, ident_sb)

                bq_sb = stat.tile([P, 8], F32, tag="bq")
                nc.sync.dma_start(bq_sb[:], bqs[l])
                bk_sb = stat.tile([P, 8], F32, tag="bk")
                nc.sync.dma_start(bk_sb[:], bks[l])
                b1_sb = stat.tile([P, 32], F32, tag="b1")
                nc.sync.dma_start(b1_sb[:], b1s[l])

                # ---- kT[hp] = wk_hp^T @ h -> DRAM kstage (fp32r) ----
                for hp in range(8):
                    wkc = wstr.tile([P, 8, P], F32, tag="wkcol")
                    nc.sync.dma_start(wkc[:], wk[l, hp])
                    for half in range(2):
                        ps = psmm.tile([P, 512], F32, tag="mm")
                        for cc in range(8):
                            nc.tensor.matmul(
                                ps[:], wkc[:, cc, :],
                                hT[:, cc, half * 512:(half + 1) * 512],
                                start=(cc == 0), stop=(cc == 7),
                            )
                        kt = outst.tile([P, 512], F32, tag="ktmp", bufs=2)
                        nc.vector.tensor_scalar(
                            out=kt[:], in0=ps[:],
                            scalar1=bk_sb[:, hp:hp + 1], scalar2=None,
                            op0=ALU.add,
                        )
                        nc.sync.dma_start(
                            kstage[hp, :, half * 512:(half + 1) * 512], kt[:])

                # ---- qT[hp] = wq_hp^T @ h_mine (fp32r) ----
                for hp in range(8):
                    wqc = wstr.tile([P, 8, P], F32, tag="wkcol")
                    nc.sync.dma_start(wqc[:], wq[l, hp])
                    ps = psmm.tile([P, 2 * P], F32, tag="mm")
                    for cc in range(8):
                        nc.tensor.matmul(
                            ps[:], wqc[:, cc, :], hTm[:, cc, :],
                            start=(cc == 0), stop=(cc == 7),
                        )
                    nc.vector.tensor_scalar(
                        out=qT[:, hp, :], in0=ps[:],
                        scalar1=bq_sb[:, hp:hp + 1], scalar2=None, op0=ALU.add,
                    )

                # ---- v[gb] = h[gb] @ wv -> DRAM vstage (fp32r) ----
                for half in range(2):
                    wvh = [wres.tile([P, 512], F32R, tag=f"wvh{cc}",
                                     name=f"wvh{cc}", bufs=1)
                           for cc in range(8)]
                    for cc in range(8):
                        nc.sync.dma_start(
                            wvh[cc][:],
                            wv[l, cc, :, half * 512:(half + 1) * 512])
                    for gb in range(NB):
                        ps = psmm.tile([P, 512], F32, tag="mm")
                        for cc in range(8):
                            nc.tensor.matmul(
                                ps[:], hTr[:, cc, gb * P:(gb + 1) * P],
                                wvh[cc][:],
                                start=(cc == 0), stop=(cc == 7),
                            )
                        vt = outst.tile([P, 512], F32, tag="vtmp", bufs=1)
                        nc.vector.tensor_copy(vt[:], ps[:])
                        nc.sync.dma_start(
                            vstage[gb, :, half * 512:(half + 1) * 512], vt[:])

                # ---- attention (scores fp32r, o fp32r N=128) ----
                for hp in range(8):
                    ksl = attn.tile([P, 8 * P], F32, tag="ksl")
                    nc.sync.dma_start(ksl[:], kstage[hp])
                    vsl = attn.tile([P, NB, P], F32, tag="vsl", bufs=1)
                    nc.sync.dma_start(
                        vsl[:],
                        vstage[:, :, hp * P:(hp + 1) * P]
                        .rearrange("a p c -> p a c"))
                    for si, (nks, mask_sb, spool) in enumerate(
                            ((4, maskA_sb, psmm), (8, maskB_sb, psB))):
                        ot = psot.tile([P, P], F32, tag="ot")
                        for h01 in range(2):
                            dlo = h01 * HS
                            sc = spool.tile([P, nks * P], F32,
                                            tag="mm" if si == 0 else "sc1")
                            for j in range(nks // 4):
                                nc.tensor.matmul(
                                    sc[:, j * 512:(j + 1) * 512],
                                    qT[dlo:dlo + HS, hp, si * P:(si + 1) * P],
                                    ksl[dlo:dlo + HS, j * 512:(j + 1) * 512],
                                    start=True, stop=True,
                                )
                            nc.vector.tensor_tensor(
                                out=sc[:], in0=sc[:],
                                in1=mask_sb[:, :nks * P], op=ALU.add)
                            nmx = stat.tile([P, 1], F32, tag="nmx")
                            nc.vector.tensor_reduce(
                                out=nmx[:], in_=sc[:], axis=AX.X, op=ALU.max,
                                negate=True)
                            pexp = attn.tile([P, 8 * P], F32, tag="pexp")
                            sume = stat.tile([P, 1], F32, tag="sume")
                            nc.scalar.activation(
                                pexp[:, :nks * P], sc[:], AF.Exp,
                                bias=nmx[:], accum_out=sume[:])
                            rsum = stat.tile([P, 1], F32, tag="rsum")
                            nc.vector.reciprocal(rsum[:], sume[:])
                            nc.vector.tensor_scalar(
                                out=pexp[:, :nks * P], in0=pexp[:, :nks * P],
                                scalar1=rsum[:], scalar2=None, op0=ALU.mult)
                            for j in range(nks):
                                ptp = pstr.tile([P, P], F32, tag="tp")
                                nc.tensor.transpose(
                                    ptp[:], pexp[:, j * P:(j + 1) * P],
                                    ident_sb[:])
                                pts = attn.tile([P, P], F32, tag="pts")
                                nc.vector.tensor_copy(pts[:], ptp[:])
                                nc.tensor.matmul(
                                    ot[dlo:dlo + HS, :],
                                    vsl[:, j, dlo:dlo + HS],
                                    pts[:],
                                    start=(j == 0), stop=(j == nks - 1),
                                )
                        nc.scalar.copy(oT[:, hp, si * P:(si + 1) * P], ot[:])

                # ---- out-proj + residual (fp32, wo streamed once) ----
                for half in range(2):
                    psop = [psmm.tile([P, 512], F32, tag="mm",
                                      name=f"psop{half}_{i}")
                            for i in range(2)]
                    for hp in range(8):
                        wot = wstr.tile([P, 512], F32, tag="wot")
                        nc.sync.dma_start(
                            wot[:],
                            wo[l, hp, :, half * 512:(half + 1) * 512])
                        for i in range(2):
                            nc.tensor.matmul(
                                psop[i][:], oT[:, hp, i * P:(i + 1) * P],
                                wot[:],
                                start=(hp == 0), stop=(hp == 7),
                            )
                    for i in range(2):
                        xsl = xlocal[i][:, half * 512:(half + 1) * 512]
                        nc.vector.tensor_add(xsl, psop[i][:], xsl)

                # ---- LN2 -> h2T ----
                for i in range(2):
                    h2 = xstr.tile([P, C], F32, tag="lnout")
                    layernorm(xlocal[i][:], h2[:])
                    for cc in range(8):
                        transpose_block(h2T[:, cc, i * P:(i + 1) * P],
                                        h2[:, cc * P:(cc + 1) * P],
                                        F32, ident_sb)

                # ---- FFN1 -> DRAM ffstage (fp32r) ----
                for fb in range(32):
                    w1c = wstr.tile([P, 8, P], F32R, tag="w1c")
                    nc.sync.dma_start(w1c[:], w1[l, fb])
                    ps = psmm.tile([P, 2 * P], F32, tag="mm")
                    for cc in range(8):
                        nc.tensor.matmul(
                            ps[:], w1c[:, cc, :], h2T[:, cc, :],
                            start=(cc == 0), stop=(cc == 7),
                        )
                    ft = outst.tile([P, 2 * P], F32R, tag="ftmp", bufs=2)
                    nc.scalar.activation(
                        ft[:], ps[:], AF.Relu, bias=b1_sb[:, fb:fb + 1])
                    nc.sync.dma_start(ffstage[fb], ft[:])

                # ---- FFN2 + residual (fp32r, w2 loaded once) ----
                for half in range(2):
                    psff = [psmm.tile([P, 512], F32, tag="mm",
                                      name=f"psff{half}_{i}")
                            for i in range(2)]
                    for fb in range(32):
                        w2t = wstr.tile([P, 512], F32R, tag="w2t")
                        nc.sync.dma_start(
                            w2t[:],
                            w2[l, fb, :, half * 512:(half + 1) * 512])
                        fft = attn.tile([P, 2 * P], F32R, tag="fft", bufs=1)
                        nc.sync.dma_start(fft[:], ffstage[fb])
                        for i in range(2):
                            nc.tensor.matmul(
                                psff[i][:], fft[:, i * P:(i + 1) * P],
                                w2t[:],
                                start=(fb == 0), stop=(fb == 31),
                            )
                    for i in range(2):
                        xsl = xlocal[i][:, half * 512:(half + 1) * 512]
                        nc.vector.tensor_add(xsl, psff[i][:], xsl)

                # ---- AllGather x ----
                ag_in = dpool.tile([2, P, C], F32, tag="agin")
                nc.sync.dma_start(ag_in[0], xlocal[0][:])
                nc.sync.dma_start(ag_in[1], xlocal[1][:])
                ag_out = dpool.tile([NB, P, C], F32, tag="agout")
                nc.gpsimd.collective_compute(
                    "AllGather", mybir.AluOpType.bypass,
                    replica_groups=[[0, 1, 2, 3], [4, 5, 6, 7]],
                    ins=[ag_in.opt()],
                    outs=[ag_out.opt()],
                )

            # ---- final LN + lm_head (bf16 full-rate GEMM) ----
            xfT = pers.tile([P, 8, NB * P], BF16, tag="hT")
            for rb in range(NB):
                xr = xstr.tile([P, C], F32, tag="xrep")
                nc.sync.dma_start(xr[:], ag_out[rb])
                xf = xstr.tile([P, C], BF16, tag="xfin")
                layernorm(xr[:], xf[:])
                for cc in range(8):
                    transpose_block(xfT[:, cc, rb * P:(rb + 1) * P],
                                    xf[:, cc * P:(cc + 1) * P],
                                    BF16, identB_sb)

            for ch in range(NCHUNK):
                off = ch * VCH
                lm_sb = [wres.tile([P, VCH], BF16, tag=f"wvh{cc}",
                                   name=f"lmw{cc}_{ch}", bufs=1)
                         for cc in range(8)]
                for cc in range(8):
                    nc.sync.dma_start(lm_sb[cc][:], lmw[cc][:, off:off + VCH])
                lmb_sb = outst.tile([P, VCH], F32, tag="lo", name="lmb_sb", bufs=2)
                nc.sync.dma_start(lmb_sb[:], lmbr[:, off:off + VCH])
                for rb in range(NB):
                    ps = psmm.tile([P, VCH], F32, tag="mm")
                    for cc in range(8):
                        nc.tensor.matmul(
                            ps[:], xfT[:, cc, rb * P:(rb + 1) * P],
                            lm_sb[cc][:], start=(cc == 0), stop=(cc == 7),
                        )
                    lo = outst.tile([P, VCH], F32, tag="lo", bufs=2)
                    nc.vector.tensor_add(lo[:], ps[:], lmb_sb[:])
                    nc.sync.dma_start(logits[rb, :, off:off + VCH], lo[:])

    nc.compile()
    return nc


def _prep_inputs(idx, tok_emb, pos_emb, wq, bq, wk, bk, wv, bv, wo, bo,
                 ln1g, ln1b, w1, b1, w2, b2, ln2g, ln2b, lnfg, lnfb, lmw, lmb):
    import ml_dtypes
    BF = ml_dtypes.bfloat16
    f = lambda a: np.asarray(a, dtype=np.float32)
    idx = np.asarray(idx)
    tok_emb, pos_emb = f(tok_emb), f(pos_emb)

    x0_full = tok_emb[idx] + pos_emb[None, :T, :]
    x0p = np.empty((B, NB, P, C), np.float32)
    for bpos, s in enumerate(PERM):
        x0p[:, bpos] = x0_full[:, s * P:(s + 1) * P, :]

    wq2 = f(wq).transpose(0, 2, 1, 3).reshape(L, C, C)
    wk2 = f(wk).transpose(0, 2, 1, 3).reshape(L, C, C)
    wv2 = f(wv).transpose(0, 2, 1, 3).reshape(L, C, C)
    bq2, bk2, bv2 = [f(b).reshape(L, C) for b in (bq, bk, bv)]
    g1 = f(ln1g)[:, :, None]
    wq_eff = g1 * wq2 * SCALE
    wk_eff = g1 * wk2
    wv_eff = g1 * wv2
    bq_eff = np.einsum('lc,lcd->ld', f(ln1b), wq2 * SCALE) + bq2 * SCALE
    bk_eff = np.einsum('lc,lcd->ld', f(ln1b), wk2) + bk2
    wo_ = f(wo)
    w1f = f(w1)
    w1_eff = f(ln2g)[:, :, None] * w1f
    b1_eff = np.einsum('lc,lcf->lf', f(ln2b), w1f) + f(b1)
    w2_ = f(w2)
    lmw_eff = f(lnfg)[:, None] * f(lmw)
    lmb_eff = f(lnfb) @ f(lmw) + f(lmb)

    def kxm(w):  # [L, C, N] -> [L, 8, 128, N]
        return np.ascontiguousarray(w.reshape(L, 8, P, w.shape[-1]))

    def qkgrp(w):  # [L, C, C] -> [L, hp, p, cc, pcol] for contiguous loads
        return np.ascontiguousarray(
            w.reshape(L, 8, P, 8, P).transpose(0, 3, 2, 1, 4))

    w1g = np.ascontiguousarray(
        w1_eff.reshape(L, 8, P, 32, P).transpose(0, 3, 2, 1, 4))

    shared = {
        "wq": qkgrp(wq_eff), "wk": qkgrp(wk_eff), "wv": kxm(wv_eff),
        "wo": kxm(wo_),
        "w1": w1g,
        "w2": np.ascontiguousarray(w2_.reshape(L, 32, P, C)),
        "bqs": np.ascontiguousarray(bq_eff.reshape(L, 8, P).transpose(0, 2, 1)),
        "bks": np.ascontiguousarray(bk_eff.reshape(L, 8, P).transpose(0, 2, 1)),
        "b1s": np.ascontiguousarray(b1_eff.reshape(L, 32, P).transpose(0, 2, 1)),
        "ident": np.eye(P, dtype=np.float32),
        "identB": np.eye(P, dtype=np.float32).astype(BF),
    }

    tri = np.triu(np.full((P, P), NEG, np.float32), 1)
    lmw8 = lmw_eff.reshape(8, P, V)
    in_maps = []
    for r in range(NCORES):
        g, q = divmod(r, GSIZE)
        sA, sB = q, 7 - q
        mA = np.zeros((P, 4 * P), np.float32)
        mB = np.zeros((P, 8 * P), np.float32)
        for j in range(4):
            if j > sA:
                mA[:, j * P:(j + 1) * P] = NEG
            elif j == sA:
                mA[:, j * P:(j + 1) * P] = tri
        for j in range(8):
            if j > sB:
                mB[:, j * P:(j + 1) * P] = NEG
            elif j == sB:
                mB[:, j * P:(j + 1) * P] = tri
        m = dict(shared)
        m["maskA"] = mA
        m["maskB"] = mB
        m["x0"] = np.ascontiguousarray(x0p[g])
        m["xloc0"] = np.ascontiguousarray(x0p[g, 2 * q:2 * q + 2])
        m["lmw"] = np.ascontiguousarray(lmw8[:, :, q * VSH:(q + 1) * VSH]
                                        .astype(BF))
        m["lmbr"] = np.ascontiguousarray(np.broadcast_to(
            lmb_eff[q * VSH:(q + 1) * VSH][None, :], (P, VSH)))
        in_maps.append(m)
    return in_maps


def _assemble(results):
    out = np.empty((B, T, V), np.float32)
    for r in range(NCORES):
        g, q = divmod(r, GSIZE)
        lg = results[r]["logits"]
        for bpos, s in enumerate(PERM):
            out[g, s * P:(s + 1) * P, q * VSH:(q + 1) * VSH] = lg[bpos]
    return out


def run(inputs, trace=False):
    from concourse.bass_utils import run_bass_kernel_spmd

    if "nc" not in _CACHE:
        _CACHE["nc"] = _build_program()
    in_maps = _prep_inputs(**inputs)
    res = run_bass_kernel_spmd(
        _CACHE["nc"], in_maps, list(range(NCORES)), trace=trace)
    return _assemble(res.results), res


def kernel(**inputs) -> np.ndarray:
    out, _ = run(inputs, trace=False)
    return out
